# revision 2
# baseline (speedup 1.0000x reference)
"""Trainium2 Bass kernel v3 for nn_BASE_49821620633700 (sparse_attention).

Replicated on all 8 NeuronCores (collectives measured at 80-95us on this
fleet vs a 15us NEFF floor; replication is free).  Host reads core 0.

Structure (single core):
  * inputs host-packed into 7 DRAM blobs, issued big-first across the two
    HWDGE rings in priority order (consts, xtA, mP | xnP, xbP, w2P).
  * PE warmup matmuls + ACT table preloads (Relu/Sigmoid/Exp/Sqrt/Square)
    during the DMA phase: HAM clock gate opens, no mid-kernel 1.3us
    ACT_TABLE_LOADs.
  * SE pooling on TensorE from x^T tiles (ones-lhsT matmuls + transposes)
    instead of a 4.4us DVE reduce.
  * scores processed in PAIRS of 64-query blocks (one (64,256) psum), the
    softmax chain split across GpSimd (mask-add, weight-scale) / ACT (exp)
    / DVE (reduce, corr, recip); selection-matrix transposes put each
    softmax weight at its final (parity, pair-index) slot so the value
    matmuls emit parity-major f-tiles straight into PSUM.
  * SE gate folded into the O_A copy and f-tile casts (per-column mult);
    merge is one tensor_tensor_reduce per (m,h) with stats accumulation.
  * InstanceNorm via accumulated sums/sumsq; LeakyReLU as max(x, 0.2x)
    (the ACT Lrelu alpha is broken in this toolchain).
"""
import sys

if "/opt/trn_rl_repo" not in sys.path:
    sys.path.insert(0, "/opt/trn_rl_repo")

import numpy as np
import concourse.bass as bass
import concourse.mybir as mybir
from concourse import tile
from concourse.bass_utils import run_bass_kernel_spmd

F32 = mybir.dt.float32
BF16 = mybir.dt.bfloat16
F8E3 = mybir.dt.float8e3
AF = mybir.ActivationFunctionType
ALU = mybir.AluOpType

H = W = 32
HW = H * W
C = 512
R_SE = C // 16
EPS = 1e-5
KC = C // 128
MASKVAL = -100.0 * C
NCORES = 8

FP8_M = False       # M-fold matrices in fp8 e3m4 (lhsT) against bf16 rhs
MSCL = 256.0        # prescale on both down-conv halves (cancelled by the norm)

# constF column layout (f32, 128 partitions)
C_SW1 = 0            # (128, 128)  se_w1T/HW, 4 chunks of 32
C_SEL01 = 128        # parts 0:64   sel0 | sel1  (64, 256 each)
C_SW2 = 128          # parts 64:96  sw2 (32, 512)
C_SEL23 = 640        # parts 0:64   sel2 | sel3
C_B2 = 640           # part 64      b2 (1, 512)
C_MASK = 1152        # parts 0:64   maskx4 (64, 512)
C_B2C = 1664         # (128, 4)
C_CORR = 1668        # parts 0:64   corr4 (64, 4)
C_B1 = 1672          # parts 64:96  b1 (32, 1)
CONSTF_COLS = 1673


def gussin_np(v=1.5, n=32):
    d = (np.arange(n)[:, None] - np.arange(n)[None, :]).astype(np.float64) ** 2
    g = np.exp(-(d[:, None, :, None] + d[None, :, None, :]) / (2.0 * v * v)) / (
        2.0 * np.pi * v * v
    )
    g = g.reshape(n * n, n, n)
    return (g / g.sum((-1, -2), keepdims=True)).astype(np.float32)


def _bf16(a):
    import ml_dtypes

    return np.ascontiguousarray(a).astype(ml_dtypes.bfloat16)


def _f8e3(a):
    import ml_dtypes

    return np.ascontiguousarray(a).astype(ml_dtypes.float8_e3m4)


def _sel_mats():
    """sel_j (64, 256) for j = s%4: query (u, cq) (wn row 32u+cq) has parity
    h = cq%2 and f-row r = 32j + 16u + (cq-h)/2; lands at column 128h + r."""
    sels = []
    for j in range(4):
        m = np.zeros((64, 256), np.float32)
        for u in range(2):
            for h in range(2):
                for t in range(16):
                    q = 32 * u + h + 2 * t
                    m[q, 128 * h + 32 * j + 16 * u + t] = 1.0
        sels.append(m)
    return sels


def _mask_corr():
    u = np.arange(2)[:, None, None, None]
    c = np.arange(W)[None, :, None, None]
    i = np.arange(4)[None, None, :, None]
    qc = np.arange(W)[None, None, None, :]
    sel = (i >= u) & (i <= u + 2) & (np.abs(qc - c) <= 1)
    maskbig = np.where(sel, 0.0, MASKVAL).reshape(64, 128).astype(np.float32)
    corr = np.where((np.arange(W) % W) % 31 == 0, 3.0, 0.0)
    corr = np.tile(corr, 2).reshape(64, 1).astype(np.float32)
    return maskbig, corr


def prep_inputs(x, se_w1, se_b1, se_w2, se_b2, down_w):
    x = np.asarray(x, np.float32)
    xn = np.ascontiguousarray(x.reshape(C, HW))
    xT = xn.T

    xnP = xn.reshape(KC, 128, HW).transpose(1, 0, 2).reshape(128, KC * HW)
    xtA = xT.reshape(8, 128, C).transpose(1, 0, 2).reshape(128, 8 * C)

    xb = np.zeros((16, 128, C), np.float32)
    for s in range(16):
        lo, hi = 64 * s - 32, 64 * s + 96
        slo, shi = max(lo, 0), min(hi, HW)
        xb[s, slo - lo:shi - lo] = xT[slo:shi]
    xbP = xb.transpose(1, 0, 2).reshape(128, 16 * C)

    gus = gussin_np(1.5, H).reshape(HW, HW)
    w1 = np.asarray(down_w, np.float32)[:, :C] * MSCL
    mP = np.zeros((128, 2 * 8 * C), np.float32)
    for h in range(2):
        mT = (w1 @ gus[h::2]).T
        mP[:, h * 8 * C:(h + 1) * 8 * C] = (
            mT.reshape(8, 128, C).transpose(1, 0, 2).reshape(128, 8 * C)
        )

    w2T = np.asarray(down_w, np.float32)[:, C:].T * MSCL
    w2P = w2T.reshape(4, 128, C).transpose(1, 0, 2).reshape(128, 4 * C)

    sw1 = np.asarray(se_w1, np.float32).T / HW
    maskbig, corr = _mask_corr()
    sels = _sel_mats()
    constF = np.zeros((128, CONSTF_COLS), np.float32)
    constF[:, C_SW1:C_SW1 + 128] = (
        sw1.reshape(4, 128, R_SE).transpose(1, 0, 2).reshape(128, 4 * R_SE)
    )
    constF[0:64, C_SEL01:C_SEL01 + 256] = sels[0]
    constF[0:64, C_SEL01 + 256:C_SEL01 + 512] = sels[1]
    constF[0:64, C_SEL23:C_SEL23 + 256] = sels[2]
    constF[0:64, C_SEL23 + 256:C_SEL23 + 512] = sels[3]
    constF[64:96, C_SW2:C_SW2 + 512] = np.asarray(se_w2, np.float32).T
    constF[64:65, C_B2:C_B2 + 512] = np.asarray(se_b2, np.float32).reshape(1, C)
    for r in range(4):
        constF[0:64, C_MASK + 128 * r:C_MASK + 128 * (r + 1)] = maskbig
        constF[0:64, C_CORR + r:C_CORR + r + 1] = corr
    constF[:, C_B2C:C_B2C + 4] = np.asarray(se_b2, np.float32).reshape(KC, 128).T
    constF[64:96, C_B1:C_B1 + 1] = np.asarray(se_b1, np.float32).reshape(R_SE, 1)

    return {
        "xnP": _bf16(xnP),
        "xtA": _bf16(xtA),
        "xbP": _bf16(xbP),
        "mP": _f8e3(mP) if FP8_M else _bf16(mP),
        "w2P": _bf16(w2P),
        "constF": constF,
    }


def build_nc():
    nc = bass.Bass(target_bir_lowering=False, debug=False)

    MDT = F8E3 if FP8_M else BF16
    xnP_d = nc.declare_dram_parameter("xnP", [128, KC * HW], BF16, isOutput=False)
    xtA_d = nc.declare_dram_parameter("xtA", [128, 8 * C], BF16, isOutput=False)
    xbP_d = nc.declare_dram_parameter("xbP", [128, 16 * C], BF16, isOutput=False)
    mP_d = nc.declare_dram_parameter("mP", [128, 16 * C], MDT, isOutput=False)
    w2P_d = nc.declare_dram_parameter("w2P", [128, 4 * C], BF16, isOutput=False)
    constF_d = nc.declare_dram_parameter("constF", [128, CONSTF_COLS], F32, isOutput=False)
    out_d = nc.declare_dram_parameter("out", [128, KC * HW], BF16, isOutput=True)
    scrB_d = nc.dram_tensor("scrB", [1, 64], BF16)
    scrF_d = nc.dram_tensor("scrF", [1, 64], F32)
    scrM_d = nc.dram_tensor("scrM", [1, 64], MDT)

    with tile.TileContext(nc) as tc:
        with (
            tc.tile_pool(name="const", bufs=1) as constp,
            tc.tile_pool(name="big", bufs=1) as bigp,
            tc.tile_pool(name="work", bufs=3) as workp,
            tc.tile_pool(name="ps", bufs=2, space="PSUM") as ps,
        ):
            # ---------- DMA triggers, phase-gated (in-queue transfers are
            # packet-round-robin, so completion order is controlled by
            # blocking each queue's sequencer on the previous phase via a
            # tiny SBUF->DRAM dummy DMA that waits on the landed region) ----
            const_sb = constp.tile([128, CONSTF_COLS], F32, tag="constF", name="const_sb")
            xtA_sb = bigp.tile([128, 8 * C], BF16, tag="xtA", name="xtA_sb")
            mP_sb = bigp.tile([128, 16 * C], MDT, tag="mP", name="mP_sb")
            xn_sb = bigp.tile([128, KC * HW], BF16, tag="xn", name="xn_sb")
            xb_sb = bigp.tile([128, 16 * C], BF16, tag="xb", name="xb_sb")
            w2_sb = bigp.tile([128, 4 * C], BF16, tag="w2", name="w2_sb")

            # P0: first half of xtA | first half of mP-h0
            nc.sync.dma_start(out=xtA_sb[:, 0:4 * C], in_=xtA_d[:, 0:4 * C])
            nc.scalar.dma_start(out=mP_sb[:, 0:4 * C], in_=mP_d[:, 0:4 * C])
            nc.sync.dma_start(out=scrB_d[0:1, 0:2], in_=xtA_sb[0:1, 4 * C - 2:4 * C])
            nc.scalar.dma_start(out=scrM_d[0:1, 2:4], in_=mP_sb[0:1, 4 * C - 2:4 * C])
            # P1: second halves
            nc.sync.dma_start(out=xtA_sb[:, 4 * C:8 * C], in_=xtA_d[:, 4 * C:8 * C])
            nc.scalar.dma_start(out=mP_sb[:, 4 * C:8 * C], in_=mP_d[:, 4 * C:8 * C])
            nc.sync.dma_start(out=scrB_d[0:1, 4:6], in_=xtA_sb[0:1, 8 * C - 2:8 * C])
            nc.scalar.dma_start(out=scrM_d[0:1, 6:8], in_=mP_sb[0:1, 8 * C - 2:8 * C])
            # P2: constants | mP-h1
            nc.sync.dma_start(out=const_sb[:], in_=constF_d[:])
            nc.scalar.dma_start(out=mP_sb[:, 8 * C:16 * C], in_=mP_d[:, 8 * C:16 * C])
            nc.sync.dma_start(out=scrF_d[0:1, 8:10], in_=const_sb[0:1, 0:2])
            nc.scalar.dma_start(out=scrM_d[0:1, 10:12], in_=mP_sb[0:1, 16 * C - 2:16 * C])
            # P3: xnP | first half of xbP
            nc.sync.dma_start(out=xn_sb[:], in_=xnP_d[:])
            nc.scalar.dma_start(out=xb_sb[:, 0:8 * C], in_=xbP_d[:, 0:8 * C])
            nc.sync.dma_start(out=scrB_d[0:1, 12:14], in_=xn_sb[0:1, KC * HW - 2:KC * HW])
            nc.scalar.dma_start(out=scrB_d[0:1, 14:16], in_=xb_sb[0:1, 8 * C - 2:8 * C])
            # P4: second half of xbP | w2P
            nc.sync.dma_start(out=xb_sb[:, 8 * C:16 * C], in_=xbP_d[:, 8 * C:16 * C])
            nc.scalar.dma_start(out=w2_sb[:], in_=w2P_d[:])

            sw1 = [const_sb[:, C_SW1 + R_SE * k:C_SW1 + R_SE * (k + 1)] for k in range(KC)]
            sel_ap = [
                const_sb[0:64, C_SEL01:C_SEL01 + 256],
                const_sb[0:64, C_SEL01 + 256:C_SEL01 + 512],
                const_sb[0:64, C_SEL23:C_SEL23 + 256],
                const_sb[0:64, C_SEL23 + 256:C_SEL23 + 512],
            ]
            sw2_ap = const_sb[64:96, C_SW2:C_SW2 + 512]
            b2_ap = const_sb[64:65, C_B2:C_B2 + 512]
            maskx4_ap = const_sb[0:64, C_MASK:C_MASK + 512]
            b2c_ap = const_sb[:, C_B2C:C_B2C + 4]
            corr4_ap = const_sb[0:64, C_CORR:C_CORR + 4]
            b1_ap = const_sb[64:96, C_B1:C_B1 + 1]

            # ---------- constants + warmup + table preloads ----------
            ones_row = constp.tile([1, 128], F32, tag="ones_row", name="ones_row")
            nc.vector.memset(ones_row[:], 1.0)
            onescol = constp.tile([128, 1], BF16, tag="onescol", name="onescol")
            nc.vector.memset(onescol[:], 1.0)
            eps_sb = constp.tile([128, 1], F32, tag="eps", name="eps_sb")
            nc.vector.memset(eps_sb[:], EPS)
            zwarm = constp.tile([128, 256], BF16, tag="zwarm", name="zwarm")
            nc.vector.memset(zwarm[:], 0.0)
            junk1 = workp.tile([128, 1], F32, tag="junk1", bufs=1, name="junk1")
            for fn in (AF.Relu, AF.Sigmoid, AF.Exp):
                nc.scalar.activation(junk1[:], eps_sb[:], fn)

            wps = ps.tile([128, 256], F32, tag="f_ps", name="warm_ps")
            for i in range(12):
                nc.tensor.matmul(wps[:], zwarm[:, 0:128], zwarm[:], start=True, stop=True)

            scm_sb = [
                bigp.tile([128, HW + 64], BF16, tag=f"scm{k}", name=f"scm{k}")
                for k in range(KC)
            ]
            for k in range(KC):
                nc.gpsimd.memset(scm_sb[k][:, 0:32], 0.0)
                nc.gpsimd.memset(scm_sb[k][:, 32 + HW:64 + HW], 0.0)

            # ---------- SE (PSUM tiles borrowed from the main tags) ----------
            pool_ps = ps.tile([1, C], F32, tag="sc_ps", name="pool_ps")
            for w in range(8):
                nc.tensor.matmul(
                    pool_ps[:], onescol[:], xtA_sb[:, C * w:C * (w + 1)],
                    start=(w == 0), stop=(w == 7),
                )
            pool_sb = workp.tile([1, C], F32, tag="pool", name="pool_sb")
            nc.vector.tensor_copy(pool_sb[:], pool_ps[:])
            poolc_ps = ps.tile([128, KC], F32, tag="sc_ps", name="poolc_ps")
            for k in range(KC):
                nc.tensor.transpose(
                    poolc_ps[:, k:k + 1],
                    pool_sb[0:1, 128 * k:128 * (k + 1)],
                    ones_row[0:1, 0:1],
                )
            poolc_sb = workp.tile([128, KC], F32, tag="poolc", name="poolc_sb")
            nc.vector.tensor_copy(poolc_sb[:], poolc_ps[:])

            y1_ps = ps.tile([R_SE, 1], F32, tag="eT_ps", name="y1_ps")
            for k in range(KC):
                nc.tensor.matmul(
                    y1_ps[:], sw1[k], poolc_sb[:, k:k + 1],
                    start=(k == 0), stop=(k == KC - 1),
                )
            y1t = workp.tile([128, 1], F32, tag="y1_sb", name="y1t")
            y1_sb = y1t[64:96, :]
            nc.scalar.activation(y1_sb, y1_ps[:], AF.Relu, bias=b1_ap)

            y2_ps = ps.tile([1, C], F32, tag="eT_ps", name="y2_ps")
            nc.tensor.matmul(y2_ps[:], y1_sb, sw2_ap, start=True, stop=True)
            y2pb = workp.tile([1, C], F32, tag="y2pb", name="y2pb")
            nc.vector.tensor_tensor(out=y2pb[:], in0=y2_ps[:], in1=b2_ap, op=ALU.add)
            y2_sb = workp.tile([1, C], F32, tag="y2s", name="y2_sb")
            nc.scalar.activation(y2_sb[:], y2pb[:], AF.Sigmoid)

            ybc_ps = ps.tile([128, C], F32, tag="oaop", name="ybc_ps")
            nc.tensor.matmul(ybc_ps[:], ones_row[:], y2_sb[:], start=True, stop=True)
            ybc_sb = bigp.tile([128, C], F32, tag="ybc_sb", name="ybc_sb")
            nc.vector.tensor_copy(ybc_sb[:], ybc_ps[:])

            y2c_ps = ps.tile([128, KC], F32, tag="oaop", name="y2c_ps")
            for k in range(KC):
                nc.tensor.matmul(
                    y2c_ps[:, k:k + 1], sw2_ap[:, 128 * k:128 * (k + 1)],
                    y1_sb, start=True, stop=True,
                )
            y2cb = workp.tile([128, KC], F32, tag="y2cb", name="y2cb")
            nc.vector.tensor_tensor(out=y2cb[:], in0=y2c_ps[:], in1=b2c_ap, op=ALU.add)
            y2c_sb = workp.tile([128, KC], F32, tag="y2cs", name="y2c_sb")
            nc.scalar.activation(y2c_sb[:], y2cb[:], AF.Sigmoid)

            # ---------- S = sigmoid(gate * x), channel-major ----------
            for k in range(KC):
                nc.scalar.activation(
                    scm_sb[k][:, 32:32 + HW],
                    xn_sb[:, HW * k:HW * (k + 1)],
                    AF.Sigmoid,
                    scale=y2c_sb[:, k:k + 1],
                )

            # ---------- main pipeline ----------
            oa_sb = {}
            f_sb = {}
            eT_sb = {}
            pair_state = {}

            def emit_oa(g):
                m, h = g // 2, g % 2
                oa_ps = ps.tile([128, C], F32, tag="oaop", name=f"oa{m}_{h}")
                for w in range(8):
                    nc.tensor.matmul(
                        oa_ps[:],
                        mP_sb[:, (8 * h + w) * C + 128 * m:(8 * h + w) * C + 128 * (m + 1)],
                        xtA_sb[:, C * w:C * (w + 1)],
                        start=(w == 0), stop=(w == 7),
                    )
                oa = workp.tile([128, C], F32, tag="oa_sb", bufs=8, name=f"oas{m}_{h}")
                nc.vector.tensor_tensor(out=oa[:], in0=oa_ps[:], in1=ybc_sb[:], op=ALU.mult)
                oa_sb[(m, h)] = oa

            def emit_sc_mms(Q):
                scq = ps.tile([64, 512], F32, tag="sc_ps", name=f"sc{Q}")
                for i in range(4):
                    s = 4 * Q + i
                    for k in range(KC):
                        nc.tensor.matmul(
                            lhsT=scm_sb[k][:, 32 + 64 * s:32 + 64 * s + 64],
                            rhs=scm_sb[k][:, 64 * s:64 * s + 128],
                            out=scq[:, 128 * i:128 * (i + 1)],
                            start=(k == 0), stop=(k == KC - 1),
                        )
                pair_state[Q] = scq

            def emit_softmax(Q):
                scq = pair_state.pop(Q)
                sc2 = workp.tile([64, 512], F32, tag="sc2", name=f"sc2_{Q}")
                nc.vector.tensor_tensor(out=sc2[:], in0=scq[:], in1=maskx4_ap, op=ALU.add)
                e = workp.tile([64, 512], F32, tag="e", name=f"e{Q}")
                nc.scalar.activation(e[:], sc2[:], AF.Exp, scale=1.0 / C)
                esum = workp.tile([64, 4], F32, tag="esum", name=f"es{Q}")
                nc.vector.reduce_sum(
                    esum[:], e[:].rearrange("p (b f) -> p b f", b=4),
                    axis=mybir.AxisListType.X,
                )
                esc = workp.tile([64, 4], F32, tag="esc", name=f"esc{Q}")
                nc.vector.tensor_tensor(out=esc[:], in0=esum[:], in1=corr4_ap, op=ALU.add)
                rinv = workp.tile([64, 4], F32, tag="rinv", name=f"ri{Q}")
                nc.vector.reciprocal(rinv[:], esc[:])
                if Q == 3:
                    nc.scalar.activation(junk1[:], eps_sb[:], AF.Sqrt)
                for i in range(4):
                    s = 4 * Q + i
                    wn = workp.tile([64, 128], F32, tag="wn", bufs=4, name=f"wn{s}")
                    if i % 2 == 0:
                        nc.vector.tensor_scalar_mul(wn[:], e[:, 128 * i:128 * (i + 1)], rinv[:, i:i + 1])
                    else:
                        nc.scalar.activation(wn[:], e[:, 128 * i:128 * (i + 1)], AF.Copy, scale=rinv[:, i:i + 1])
                    eT_ps = ps.tile([128, 256], F32, tag="eT_ps", name=f"eT{s}")
                    nc.tensor.transpose(eT_ps[:], wn[:], sel_ap[s % 4])
                    et = workp.tile([128, 256], BF16, tag="eT_sb", bufs=8, name=f"eTs{s}")
                    if s % 2 == 0:
                        nc.vector.tensor_copy(et[:], eT_ps[:])
                    else:
                        nc.scalar.activation(et[:], eT_ps[:], AF.Copy)
                    eT_sb[s] = et

            def emit_value(h, tt):
                f_ps = ps.tile([128, C], F32, tag="f_ps", name=f"f{h}_{tt}")
                for j in range(4):
                    s = 4 * tt + j
                    nc.tensor.matmul(
                        f_ps[:],
                        eT_sb[s][:, 128 * h:128 * (h + 1)],
                        xb_sb[:, C * s:C * (s + 1)],
                        start=(j == 0), stop=(j == 3),
                    )
                ft = bigp.tile([128, C], BF16, tag=f"f{h}_{tt}", name=f"fs{h}_{tt}")
                nc.vector.tensor_tensor(out=ft[:], in0=f_ps[:], in1=ybc_sb[:], op=ALU.mult)
                f_sb[(h, tt)] = ft

            # g = 2*m + h; h=0 groups need only the P0/P1 mP halves
            oa_order = [0, 2, 4, 6, 1, 3, 5, 7]
            emit_oa(oa_order[0])
            emit_oa(oa_order[1])
            next_oa = 2
            for Q in range(4):
                emit_sc_mms(Q)
                if Q >= 1:
                    emit_softmax(Q - 1)
                    emit_value(0, Q - 1)
                    emit_value(1, Q - 1)
                for _ in range(2):
                    if next_oa < 8:
                        emit_oa(oa_order[next_oa])
                        next_oa += 1
            emit_softmax(3)
            emit_value(0, 3)
            emit_value(1, 3)

            # ---------- down conv + merge + stats ----------
            sums = workp.tile([128, 2 * KC], F32, tag="sums", name="sums")
            sqs = workp.tile([128, KC], F32, tag="sqs", name="sqs")
            o_sb = [
                bigp.tile([128, HW], F32, tag=f"o{m}", name=f"o{m}") for m in range(KC)
            ]
            for m in range(KC):
                o_ps2 = [
                    ps.tile([128, C], F32, tag="oaop", name=f"o_ps{m}_{h}")
                    for h in range(2)
                ]
                for tt in range(4):
                    for h in range(2):
                        nc.tensor.matmul(
                            o_ps2[h][:],
                            w2_sb[:, C * tt + 128 * m:C * tt + 128 * (m + 1)],
                            f_sb[(h, tt)][:],
                            start=(tt == 0), stop=(tt == 3),
                        )
                for h in range(2):
                    half = o_sb[m][:, C * h:C * (h + 1)]
                    nc.vector.scalar_tensor_tensor(
                        out=half, in0=o_ps2[h][:], scalar=1.0,
                        in1=oa_sb[(m, h)][:],
                        op0=ALU.mult, op1=ALU.add,
                        accum_out=sums[:, 2 * m + h:2 * m + h + 1],
                    )
                sqjunk = workp.tile([128, HW], F32, tag="sqjunk", name=f"sq{m}")
                nc.scalar.activation(
                    sqjunk[:], o_sb[m][:], AF.Square,
                    accum_out=sqs[:, m:m + 1],
                )

            # ---------- instance norm params ----------
            sv = sums[:].rearrange("p (m two) -> p two m", two=2)
            mean = workp.tile([128, KC], F32, tag="meanc", name="mean_b")
            nc.vector.tensor_tensor(out=mean[:], in0=sv[:, 0, :], in1=sv[:, 1, :], op=ALU.add)
            nc.vector.tensor_scalar_mul(mean[:], mean[:], 1.0 / HW)
            esqm = sqs
            msq = workp.tile([128, KC], F32, tag="msqc", name="msq_b")
            nc.vector.tensor_tensor(out=msq[:], in0=mean[:], in1=mean[:], op=ALU.mult)
            var = workp.tile([128, KC], F32, tag="varc", name="var_b")
            nc.vector.scalar_tensor_tensor(
                out=var[:], in0=esqm[:], scalar=1.0 / HW, in1=msq[:],
                op0=ALU.mult, op1=ALU.subtract,
            )
            # eps scaled by MSCL^2: var is in the prescaled domain
            std = workp.tile([128, KC], F32, tag="stdc", name="std_b")
            nc.scalar.activation(std[:], var[:], AF.Sqrt, bias=eps_sb[:], scale=1.0 / (MSCL * MSCL))
            rstdb = workp.tile([128, KC], F32, tag="rstdc", name="rstd_b")
            nc.vector.reciprocal(rstdb[:], std[:])
            nc.vector.tensor_scalar_mul(rstdb[:], rstdb[:], 1.0 / MSCL)
            nmrnb = workp.tile([128, KC], F32, tag="nmrn", name="nmrn_b")
            nc.vector.scalar_tensor_tensor(
                out=nmrnb[:], in0=mean[:], scalar=-1.0, in1=rstdb[:],
                op0=ALU.mult, op1=ALU.mult,
            )

            # ---------- normalize + LeakyReLU + out ----------
            for m in range(KC):
                t2 = workp.tile([128, HW], BF16, tag="t2", bufs=2, name=f"t2_{m}")
                nc.vector.tensor_scalar(
                    out=t2[:], in0=o_sb[m][:],
                    scalar1=rstdb[:, m:m + 1], scalar2=nmrnb[:, m:m + 1],
                    op0=ALU.mult, op1=ALU.add,
                )
                t3 = workp.tile([128, HW], BF16, tag="t3", bufs=2, name=f"t3_{m}")
                nc.scalar.activation(t3[:], t2[:], AF.Copy, scale=0.2)
                ot = workp.tile([128, HW], BF16, tag="ot", bufs=2, name=f"ot{m}")
                nc.vector.tensor_tensor(out=ot[:], in0=t2[:], in1=t3[:], op=ALU.max)
                eng = nc.sync if m % 2 == 0 else nc.scalar
                eng.dma_start(out=out_d[:, HW * m:HW * (m + 1)], in_=ot[:])

    return nc


def _split_drain_waits(nc, keep=1):
    """This walrus build allows at most 1 sync wait per instruction; hoist the
    extras onto preceding NoOps on the same engine."""
    n = 0
    for f in nc.m.functions:
        for bb in f.blocks:
            newlist = []
            for ins in bb.instructions:
                si = getattr(ins, "sync_info", None)
                if si is not None and si.on_wait and len(si.on_wait) > keep:
                    waits = list(si.on_wait)
                    for w in waits[:-keep]:
                        nop = mybir.InstNoOp(name=f"I-dw{n}", ins=[], outs=[])
                        n += 1
                        nop.engine = ins.engine
                        nop.sync_info = mybir.SyncInfo(on_wait=[w], on_update=[])
                        newlist.append(nop)
                    si.on_wait = waits[-keep:]
                newlist.append(ins)
            bb.instructions = newlist
    return n


_BUILT = None


def get_built():
    global _BUILT
    if _BUILT is None:
        nc = build_nc()
        _split_drain_waits(nc)
        _BUILT = nc
    return _BUILT


def _unpack_out(out):
    return (
        np.asarray(out).astype(np.float32)
        .reshape(128, KC, HW)
        .transpose(1, 0, 2)
        .reshape(1, C, H, W)
    )


def kernel(x, se_w1, se_b1, se_w2, se_b2, down_w, _trace=False):
    ins = prep_inputs(x, se_w1, se_b1, se_w2, se_b2, down_w)
    nc = get_built()
    in_maps = [dict(ins) for _ in range(NCORES)]
    res = run_bass_kernel_spmd(nc, in_maps, list(range(NCORES)), trace=_trace)
    full = _unpack_out(res.results[0]["out"])
    if _trace:
        return full, res
    return full


def simulate(x, se_w1, se_b1, se_w2, se_b2, down_w):
    """Numpy replay of the device program (validates packing + index math)."""
    ins = prep_inputs(x, se_w1, se_b1, se_w2, se_b2, down_w)
    xn = ins["xnP"].astype(np.float32).reshape(128, KC, HW).transpose(1, 0, 2).reshape(C, HW)
    xtA = ins["xtA"].astype(np.float32).reshape(128, 8, C).transpose(1, 0, 2).reshape(HW, C)
    xb = ins["xbP"].astype(np.float32).reshape(128, 16, C).transpose(1, 0, 2)
    mP = ins["mP"].astype(np.float32)
    w2P = ins["w2P"].astype(np.float32).reshape(128, 4, C).transpose(1, 0, 2).reshape(C, C)
    cF = ins["constF"]
    sw1 = cF[:, C_SW1:C_SW1 + 128].reshape(128, 4, R_SE).transpose(1, 0, 2).reshape(C, R_SE)
    sels = [
        cF[0:64, C_SEL01:C_SEL01 + 256],
        cF[0:64, C_SEL01 + 256:C_SEL01 + 512],
        cF[0:64, C_SEL23:C_SEL23 + 256],
        cF[0:64, C_SEL23 + 256:C_SEL23 + 512],
    ]
    sw2 = cF[64:96, C_SW2:C_SW2 + 512]
    b2 = cF[64:65, C_B2:C_B2 + 512]
    maskbig = cF[0:64, C_MASK:C_MASK + 128]
    corr = cF[0:64, C_CORR:C_CORR + 1]
    b1 = cF[64:96, C_B1:C_B1 + 1]

    pool = xtA.sum(0)[:, None]
    y1 = np.maximum(sw1.T @ pool + b1, 0)
    y2 = 1 / (1 + np.exp(-(sw2.T @ y1 + b2.T)))
    gate = y2[:, 0]

    scm = 1 / (1 + np.exp(-(gate[:, None] * xn)))
    scm_g = np.zeros((C, HW + 64), np.float32)
    scm_g[:, 32:32 + HW] = scm

    oa = {}
    for m in range(KC):
        for h in range(2):
            acc = np.zeros((128, C), np.float32)
            for w in range(8):
                lhsT = mP[:, (8 * h + w) * C + 128 * m:(8 * h + w) * C + 128 * (m + 1)]
                acc += lhsT.T @ xtA[128 * w:128 * (w + 1)]
            oa[(m, h)] = acc * gate[None, :]

    eT = {}
    for s in range(16):
        q = scm_g[:, 32 + 64 * s:32 + 64 * s + 64]
        band = scm_g[:, 64 * s:64 * s + 128]
        sc = q.T @ band + maskbig
        e = np.exp(sc / C)
        wn = e / (e.sum(1, keepdims=True) + corr)
        eT[s] = wn.T @ sels[s % 4]

    f = {}
    for tt in range(4):
        for h in range(2):
            acc = np.zeros((128, C), np.float32)
            for j in range(4):
                s = 4 * tt + j
                acc += eT[s][:, 128 * h:128 * (h + 1)].T @ xb[s]
            f[(h, tt)] = acc * gate[None, :]

    o_sb = np.zeros((KC, 128, HW), np.float32)
    for m in range(KC):
        for h in range(2):
            acc = np.zeros((128, C), np.float32)
            for tt in range(4):
                acc += w2P[128 * tt:128 * (tt + 1), 128 * m:128 * (m + 1)].T @ f[(h, tt)]
            o_sb[m][:, C * h:C * (h + 1)] = acc + oa[(m, h)]

    o = o_sb.reshape(C, HW)
    mean = o.mean(1, keepdims=True)
    var = o.var(1, keepdims=True)
    on = (o - mean) / np.sqrt(var / (MSCL * MSCL) + EPS) / MSCL
    out = np.where(on >= 0, on, 0.2 * on)
    return out.reshape(1, C, H, W)


if __name__ == "__main__":
    sys.path.insert(0, "/root/problem")
    import jax.numpy as jnp
    import reference as ref

    inputs = {k: np.asarray(v) for k, v in ref.setup_inputs().items()}
    expected = np.asarray(ref.reference(**{k: jnp.asarray(v) for k, v in inputs.items()}))
    got = simulate(**inputs)
    rel = np.linalg.norm(got - expected) / np.linalg.norm(expected)
    print(f"sim rel err: {rel:.6g}  max abs: {np.abs(got - expected).max():.6g}")
